# revision 6
# baseline (speedup 1.0000x reference)
"""Trainium2 Bass kernel for quantized 3x3 conv2d (stride 1, pad 1).

Reference computes: conv2d(quant16(x), quant16(w)) where quant16 rounds to
signed 16-bit fixed point with 12 fractional bits (round-half-even, /4096).

Strategy (per core, data-parallel over batch: 4 images/core on 8 cores):
  - Quantize on device with the magic-number trick (+1.5*2^23 in f32 RNE)
    giving rx = round(x*4096) exactly (round-half-even, matches jnp.round).
  - Single fp16 term: Xh = fp16(rx) (RNE). |rx| <= 32768 so the fp16
    rounding error is <= 16 integer ulps (~2^-11 relative), which lands the
    conv output at ~2e-4 max rel err -- far inside the 2e-2 gate. The
    second correction term (Xl) costs a full extra matmul pass and is not
    needed at this tolerance. rw = round(w*4096) fits fp16 exactly
    (|rw| ~ 1100 < 2048).
  - 3x3 conv = 9 shifted matmuls accumulating in PSUM over a zero-padded
    58x58 image laid out [Cin=128 partitions, 58*58]. Contraction dim =
    partition dim = Cin = 128. Cout=256 -> two 128-row output chunks.
  - One round = (image, cout-chunk): 9 taps x 8 PSUM banks (all 56 output
    rows). Taps outer so 8 consecutive matmuls share one stationary weight.
  - PSUM result = 2^24 * conv(qx, qw); the PSUM->SBUF eviction copy applies
    the 2^-24 scale for free (ScalarE activation Copy with scale).
"""

import numpy as np

B, CIN, COUT, H, W = 32, 128, 256, 56, 56
NCORES = 8
BL = B // NCORES          # images per core
HP = H + 2                # padded height/width (58)
NPIX = H * W              # 3136
NPAD = HP * HP            # 3364
SCALE = 4096.0
MAGIC = 12582912.0        # 1.5 * 2**23: f32 add forces round-to-nearest-even at ulp=1
OSCALE = 1.0 / (SCALE * SCALE)
GROUP_ROWS = 7            # output rows per PSUM tile
NGRP = H // GROUP_ROWS    # 8 groups of 392 px
GRP_PIX = GROUP_ROWS * W  # 392

_cache = {}


def _build():
    import concourse.bacc as bacc
    import concourse.mybir as mybir
    import concourse.tile as tile

    f32, f16 = mybir.dt.float32, mybir.dt.float16
    Copy = mybir.ActivationFunctionType.Copy
    Alu = mybir.AluOpType

    nc = bacc.Bacc("TRN2", target_bir_lowering=False)
    # x arrives zero-padded to 58x58 from the host so every DMA is contiguous
    x_in = nc.dram_tensor("x", [BL, CIN, NPAD], f32, kind="ExternalInput")
    w_in = nc.dram_tensor("w", [CIN, 9 * COUT], f32, kind="ExternalInput")
    out = nc.dram_tensor("out", [BL, COUT, NPIX], f32, kind="ExternalOutput")

    with tile.TileContext(nc) as tc:
        with (
            tc.tile_pool(name="fixed", bufs=1) as fx,
            tc.tile_pool(name="psum", bufs=1, space="PSUM") as pp,
        ):
            # ---- per-image ping-pong buffers ----
            xsts = [fx.tile([CIN, NPAD], f32, name=f"xst{i}") for i in range(2)]
            ts = [fx.tile([CIN, NPAD], f32, name=f"t{i}") for i in range(2)]
            xhs = [fx.tile([CIN, NPAD], f16, name=f"xh{i}") for i in range(2)]
            osbs = [fx.tile([128, NPIX], f32, name=f"osb{i}") for i in range(3)]
            ps = [pp.tile([128, GRP_PIX], f32, name=f"ps{i}") for i in range(8)]
            wst = fx.tile([CIN, 9 * COUT], f32)
            wt = fx.tile([CIN, 9 * COUT], f32)
            w16 = fx.tile([CIN, 9 * COUT], f16)

            # Staging is split into row-chunks so the quantize chain (and the
            # first PE matmuls) start before the whole image has landed.
            # x DMAs issue from the (otherwise idle) GpSimd queue so they
            # don't serialize behind the weight DMAs on the Sync queue.
            CHUNKS = [(0, 30), (30, HP)]

            def stage_chunk(b, c, rng=None):
                s = b % 2
                xst, t, xh = xsts[s], ts[s], xhs[s]
                r0, r1 = rng if rng is not None else CHUNKS[c]
                lo, hi = r0 * HP, r1 * HP
                nc.gpsimd.dma_start(out=xst[:, lo:hi], in_=x_in[b, :, lo:hi])
                # t = x*4096 + MAGIC  (exact fma; the add performs RNE rounding)
                nc.scalar.activation(t[:, lo:hi], xst[:, lo:hi], Copy, bias=MAGIC, scale=SCALE)
                # Xh = fp16(rx)  (f32 subtract exact, fp16 convert-on-write RNE)
                nc.vector.tensor_scalar_add(xh[:, lo:hi], t[:, lo:hi], -MAGIC)

            # ---- weights: load + quantize to fp16 integers (rw = round(w*4096)) ----
            # ch-major layout [ci, (ch, tap, co)]. Staging is choreographed so
            # only first-matmul-critical transfers are in flight early (the 16
            # DMA engines split bandwidth across everything queued): Sync
            # issues w-ch0 (taps 0-2, then 3-8); GpSimd issues x slices and
            # naturally defers the non-urgent issues behind its w-ch0b
            # quantize ops (in-order queue). DVE keeps the xh chain first.
            HW_COLS = 9 * 128  # 1152 columns per cout-half

            def quant_w(eng, lo, hi):
                eng.tensor_scalar(
                    out=wt[:, lo:hi], in0=wst[:, lo:hi],
                    scalar1=SCALE, scalar2=MAGIC,
                    op0=Alu.mult, op1=Alu.add,
                )
                eng.tensor_scalar_add(w16[:, lo:hi], wt[:, lo:hi], -MAGIC)

            # w ch0 taps 0-2: gates the first LDWEIGHTS
            nc.sync.dma_start(out=wst[:, 0:384], in_=w_in[:, 0:384])
            quant_w(nc.vector, 0, 384)
            # image 0, padded rows [0,7): all round-0 g=0 taps with dh=0
            stage_chunk(0, 0, rng=(0, 7))
            # PE warmup: the tensor engine ramps from ~2x-slow to full speed
            # over ~3us of continuous execution. Run throwaway matmuls on the
            # already-staged weight tile while x is still landing so the ramp
            # happens off the critical path. ps[7]'s first real write
            # (start=True) is WAW-ordered after these on the same queue.
            for _ in range(4):
                nc.tensor.matmul(
                    ps[7][:, 0:384], w16[:, 0:128], w16[:, 0:384],
                    start=True, stop=True,
                )
            nc.sync.dma_start(out=wst[:, 384:HW_COLS], in_=w_in[:, 384:HW_COLS])
            stage_chunk(0, 0, rng=(7, 17))
            # ch0 taps 3-8 quantize on GpSimd: keeps DVE free for xh and
            # holds back the issues queued behind it until ~when w0b lands
            quant_w(nc.gpsimd, 384, HW_COLS)
            stage_chunk(0, 0, rng=(17, 30))
            stage_chunk(0, 1)
            nc.gpsimd.dma_start(
                out=wst[:, HW_COLS : 2 * HW_COLS], in_=w_in[:, HW_COLS : 2 * HW_COLS]
            )
            quant_w(nc.vector, HW_COLS, 2 * HW_COLS)
            stage_chunk(1, 0)
            stage_chunk(1, 1)

            NRND = BL * 2
            for b in range(BL):
                if b >= 2:
                    stage_chunk(b, 0)
                    stage_chunk(b, 1)
                s = b % 2
                xh3 = xhs[s][:].rearrange("p (h w) -> p h w", h=HP)

                for ch in range(2):
                    rnd = b * 2 + ch
                    osb = osbs[rnd % 3]

                    def mm(tap, g, si):
                        dh, dw = divmod(tap, 3)
                        wsl = w16[:, ch * HW_COLS + tap * 128 : ch * HW_COLS + tap * 128 + 128]
                        r0 = g * GROUP_ROWS
                        mv = xh3[:, r0 + dh : r0 + dh + GROUP_ROWS, dw : dw + W]
                        nc.tensor.matmul(
                            ps[g][:], wsl, mv,
                            start=(si == 0), stop=(si == 8),
                        )

                    if rnd == 0 or rnd == NRND - 1:
                        # g-major. Round 0: group g only needs padded rows
                        # <7g+9, so matmuls start while the image stages.
                        # Last round: each bank finishes 9 taps early, so its
                        # evict+store overlaps the remaining banks' matmuls
                        # and the tail after the final matmul is one bank.
                        for g in range(8):
                            for tap in range(9):
                                mm(tap, g, tap)
                    else:
                        # steady state: taps outer -> 8 consecutive matmuls
                        # share one stationary weight
                        for tap in range(9):
                            for g in range(8):
                                mm(tap, g, tap)

                    if rnd == NRND - 1:
                        # per-bank evict (split ACT/DVE) + per-bank store,
                        # DMA issues alternating Sync/GpSimd so the in-order
                        # issue cost doesn't serialize the tail. The final
                        # bank splits in two so its evict and store each run
                        # on both engines/queues in parallel.
                        for g in range(7):
                            dst = osb[:, g * GRP_PIX : (g + 1) * GRP_PIX]
                            if g % 2 == 0:
                                nc.scalar.activation(dst, ps[g][:], Copy, scale=OSCALE)
                            else:
                                nc.vector.tensor_scalar_mul(dst, ps[g][:], OSCALE)
                            eng = nc.sync if g % 2 == 0 else nc.gpsimd
                            eng.dma_start(
                                out=out[
                                    b,
                                    ch * 128 : (ch + 1) * 128,
                                    g * GRP_PIX : (g + 1) * GRP_PIX,
                                ],
                                in_=dst,
                            )
                        HGP = GRP_PIX // 2  # 196
                        p0 = 7 * GRP_PIX
                        for h, eng_e, eng_d in (
                            (0, nc.scalar, nc.sync),
                            (1, nc.vector, nc.gpsimd),
                        ):
                            lo, hi = p0 + h * HGP, p0 + (h + 1) * HGP
                            dst = osb[:, lo:hi]
                            if h == 0:
                                eng_e.activation(dst, ps[7][:, 0:HGP], Copy, scale=OSCALE)
                            else:
                                eng_e.tensor_scalar_mul(dst, ps[7][:, HGP:GRP_PIX], OSCALE)
                            eng_d.dma_start(
                                out=out[b, ch * 128 : (ch + 1) * 128, lo:hi],
                                in_=dst,
                            )
                    else:
                        for g in range(8):
                            dst = osb[:, g * GRP_PIX : (g + 1) * GRP_PIX]
                            if g % 2 == 0:
                                nc.scalar.activation(dst, ps[g][:], Copy, scale=OSCALE)
                            else:
                                nc.vector.tensor_scalar_mul(dst, ps[g][:], OSCALE)
                        nc.sync.dma_start(
                            out=out[b, ch * 128 : (ch + 1) * 128, :],
                            in_=osb[:],
                        )
    nc.compile()
    return nc


def _get_nc():
    if "nc" not in _cache:
        _cache["nc"] = _build()
    return _cache["nc"]


def _maybe_install_trace_bridge():
    """Optional: bridge antenv.axon_hooks so trace=True can capture NTFF."""
    import sys
    import types

    if "antenv.axon_hooks" in sys.modules:
        return
    try:
        from trn_agent_boot.trn_boot import _ntff_profile_via_ctypes

        hook = _ntff_profile_via_ctypes("/opt/axon/libaxon_pjrt.so")
        mod = types.ModuleType("antenv.axon_hooks")
        mod.get_axon_ntff_profile_hook = lambda: hook
        mod.set_axon_ntff_profile_hook = lambda h: None
        import antenv

        sys.modules["antenv.axon_hooks"] = mod
        antenv.axon_hooks = mod
    except Exception:
        pass


def kernel(**inputs):
    import os

    from concourse.bass_utils import run_bass_kernel_spmd

    x = np.ascontiguousarray(np.asarray(inputs["x"], dtype=np.float32))
    weight = np.ascontiguousarray(np.asarray(inputs["weight"], dtype=np.float32))
    assert x.shape == (B, CIN, H, W), x.shape
    assert weight.shape == (COUT, CIN, 3, 3), weight.shape

    # [Cout, Cin, kh, kw] -> [Cin, (ch, kh kw, co128)] so each (ch, tap)
    # slice is a ready [K=ci, M=co] stationary operand, ch-major so the
    # kernel can stage the ch=0 half first.
    w_r = np.ascontiguousarray(
        weight.reshape(2, 128, CIN, 9)
        .transpose(2, 0, 3, 1)
        .reshape(CIN, 9 * COUT)
    )
    xp = np.zeros((B, CIN, HP, HP), dtype=np.float32)
    xp[:, :, 1 : 1 + H, 1 : 1 + W] = x.reshape(B, CIN, H, W)
    xp = xp.reshape(B, CIN, NPAD)
    in_maps = [
        {"x": xp[i * BL : (i + 1) * BL], "w": w_r}
        for i in range(NCORES)
    ]

    trace = bool(int(os.environ.get("KERNEL_TRACE", "0")))
    if trace:
        _maybe_install_trace_bridge()
    nc = _get_nc()
    res = run_bass_kernel_spmd(nc, in_maps, core_ids=list(range(NCORES)), trace=trace)
    _cache["exec_time_ns"] = res.exec_time_ns
    _cache["res"] = res

    outs = [res.results[i]["out"].reshape(BL, COUT, H, W) for i in range(NCORES)]
    return np.concatenate(outs, axis=0)


# revision 7
# speedup vs baseline: 1.0918x; 1.0918x over previous
"""Trainium2 Bass kernel for quantized 3x3 conv2d (stride 1, pad 1).

Reference computes: conv2d(quant16(x), quant16(w)) where quant16 rounds to
signed 16-bit fixed point with 12 fractional bits (round-half-even, /4096).

Strategy (per core, data-parallel over batch: 4 images/core on 8 cores):
  - Quantize on device with the magic-number trick (+1.5*2^23 in f32 RNE)
    giving rx = round(x*4096) exactly (round-half-even, matches jnp.round).
  - Single fp16 term: Xh = fp16(rx) (RNE). |rx| <= 32768 so the fp16
    rounding error is <= 16 integer ulps (~2^-11 relative), which lands the
    conv output at ~2e-4 max rel err -- far inside the 2e-2 gate. The
    second correction term (Xl) costs a full extra matmul pass and is not
    needed at this tolerance. rw = round(w*4096) fits fp16 exactly
    (|rw| ~ 1100 < 2048).
  - 3x3 conv = 9 shifted matmuls accumulating in PSUM over a zero-padded
    58x58 image laid out [Cin=128 partitions, 58*58]. Contraction dim =
    partition dim = Cin = 128. Cout=256 -> two 128-row output chunks.
  - One round = (image, cout-chunk): 9 taps x 8 PSUM banks (all 56 output
    rows). Taps outer so 8 consecutive matmuls share one stationary weight.
  - PSUM result = 2^24 * conv(qx, qw); the PSUM->SBUF eviction copy applies
    the 2^-24 scale for free (ScalarE activation Copy with scale).
"""

import numpy as np

B, CIN, COUT, H, W = 32, 128, 256, 56, 56
NCORES = 8
BL = B // NCORES          # images per core
HP = H + 2                # padded height/width (58)
NPIX = H * W              # 3136
NPAD = HP * HP            # 3364
SCALE = 4096.0
MAGIC = 12582912.0        # 1.5 * 2**23: f32 add forces round-to-nearest-even at ulp=1
OSCALE = 1.0 / (SCALE * SCALE)
GROUP_ROWS = 7            # output rows per PSUM tile
NGRP = H // GROUP_ROWS    # 8 groups of 392 px
GRP_PIX = GROUP_ROWS * W  # 392

_cache = {}


def _build():
    import concourse.bacc as bacc
    import concourse.mybir as mybir
    import concourse.tile as tile

    f32, f16 = mybir.dt.float32, mybir.dt.float16
    Copy = mybir.ActivationFunctionType.Copy
    Alu = mybir.AluOpType

    nc = bacc.Bacc("TRN2", target_bir_lowering=False)
    # x arrives zero-padded to 58x58 from the host so every DMA is contiguous
    x_in = nc.dram_tensor("x", [BL, CIN, NPAD], f32, kind="ExternalInput")
    w_in = nc.dram_tensor("w", [CIN, 9 * COUT], f32, kind="ExternalInput")
    out = nc.dram_tensor("out", [BL, COUT, NPIX], f32, kind="ExternalOutput")

    with tile.TileContext(nc) as tc:
        with (
            tc.tile_pool(name="fixed", bufs=1) as fx,
            tc.tile_pool(name="psum", bufs=1, space="PSUM") as pp,
        ):
            # ---- per-image ping-pong buffers ----
            xsts = [fx.tile([CIN, NPAD], f32, name=f"xst{i}") for i in range(2)]
            ts = [fx.tile([CIN, NPAD], f32, name=f"t{i}") for i in range(2)]
            xhs = [fx.tile([CIN, NPAD], f16, name=f"xh{i}") for i in range(2)]
            osbs = [fx.tile([128, NPIX], f32, name=f"osb{i}") for i in range(3)]
            ps = [pp.tile([128, GRP_PIX], f32, name=f"ps{i}") for i in range(8)]
            wst = fx.tile([CIN, 9 * COUT], f32)
            wt = fx.tile([CIN, 9 * COUT], f32)
            w16 = fx.tile([CIN, 9 * COUT], f16)

            # Staging is split into row-chunks so the quantize chain (and the
            # first PE matmuls) start before the whole image has landed.
            # x DMAs issue from the (otherwise idle) GpSimd queue so they
            # don't serialize behind the weight DMAs on the Sync queue.
            CHUNKS = [(0, 30), (30, HP)]

            def stage_chunk(b, c, rng=None):
                s = b % 2
                xst, t, xh = xsts[s], ts[s], xhs[s]
                r0, r1 = rng if rng is not None else CHUNKS[c]
                lo, hi = r0 * HP, r1 * HP
                nc.gpsimd.dma_start(out=xst[:, lo:hi], in_=x_in[b, :, lo:hi])
                # t = x*4096 + MAGIC  (exact fma; the add performs RNE rounding)
                nc.scalar.activation(t[:, lo:hi], xst[:, lo:hi], Copy, bias=MAGIC, scale=SCALE)
                # Xh = fp16(rx)  (f32 subtract exact, fp16 convert-on-write RNE)
                nc.vector.tensor_scalar_add(xh[:, lo:hi], t[:, lo:hi], -MAGIC)

            # ---- weights: load + quantize to fp16 integers (rw = round(w*4096)) ----
            # ch-major layout [ci, (ch, tap, co)]. Staging is choreographed so
            # only first-matmul-critical transfers are in flight early (the 16
            # DMA engines split bandwidth across everything queued): Sync
            # issues w-ch0 (taps 0-2, then 3-8); GpSimd issues x slices and
            # naturally defers the non-urgent issues behind its w-ch0b
            # quantize ops (in-order queue). DVE keeps the xh chain first.
            HW_COLS = 9 * 128  # 1152 columns per cout-half

            def quant_w(eng, lo, hi):
                eng.tensor_scalar(
                    out=wt[:, lo:hi], in0=wst[:, lo:hi],
                    scalar1=SCALE, scalar2=MAGIC,
                    op0=Alu.mult, op1=Alu.add,
                )
                eng.tensor_scalar_add(w16[:, lo:hi], wt[:, lo:hi], -MAGIC)

            # w ch0 taps 0-2: gates the first LDWEIGHTS
            nc.sync.dma_start(out=wst[:, 0:384], in_=w_in[:, 0:384])
            quant_w(nc.vector, 0, 384)
            # image 0, padded rows [0,7): all round-0 g=0 taps with dh=0
            stage_chunk(0, 0, rng=(0, 7))
            # PE warmup: the tensor engine ramps from ~2x-slow to full speed
            # over ~3us of continuous execution. Run throwaway matmuls on the
            # already-staged weight tile while x is still landing so the ramp
            # happens off the critical path. ps[7]'s first real write
            # (start=True) is WAW-ordered after these on the same queue.
            for _ in range(4):
                nc.tensor.matmul(
                    ps[7][:, 0:384], w16[:, 0:128], w16[:, 0:384],
                    start=True, stop=True,
                )
            nc.sync.dma_start(out=wst[:, 384:HW_COLS], in_=w_in[:, 384:HW_COLS])
            stage_chunk(0, 0, rng=(7, 17))
            # ch0 taps 3-8 quantize on DVE, enqueued after the xh(0,17) ops
            # so it can't delay them (round-0 g0 needs taps 3+ only after
            # taps 0-2 have run)
            quant_w(nc.vector, 384, HW_COLS)
            stage_chunk(0, 0, rng=(17, 30))
            stage_chunk(0, 1)
            nc.gpsimd.dma_start(
                out=wst[:, HW_COLS : 2 * HW_COLS], in_=w_in[:, HW_COLS : 2 * HW_COLS]
            )
            quant_w(nc.vector, HW_COLS, 2 * HW_COLS)
            stage_chunk(1, 0)
            stage_chunk(1, 1)

            NRND = BL * 2
            for b in range(BL):
                if b >= 2:
                    stage_chunk(b, 0)
                    stage_chunk(b, 1)
                s = b % 2
                xh3 = xhs[s][:].rearrange("p (h w) -> p h w", h=HP)

                for ch in range(2):
                    rnd = b * 2 + ch
                    osb = osbs[rnd % 3]

                    def mm(tap, g, si):
                        dh, dw = divmod(tap, 3)
                        wsl = w16[:, ch * HW_COLS + tap * 128 : ch * HW_COLS + tap * 128 + 128]
                        r0 = g * GROUP_ROWS
                        mv = xh3[:, r0 + dh : r0 + dh + GROUP_ROWS, dw : dw + W]
                        nc.tensor.matmul(
                            ps[g][:], wsl, mv,
                            start=(si == 0), stop=(si == 8),
                        )

                    if rnd == 0 or rnd == NRND - 1:
                        # g-major. Round 0: group g only needs padded rows
                        # <7g+9, so matmuls start while the image stages.
                        # Last round: each bank finishes 9 taps early, so its
                        # evict+store overlaps the remaining banks' matmuls
                        # and the tail after the final matmul is one bank.
                        for g in range(8):
                            for tap in range(9):
                                mm(tap, g, tap)
                    else:
                        # steady state: taps outer -> 8 consecutive matmuls
                        # share one stationary weight
                        for tap in range(9):
                            for g in range(8):
                                mm(tap, g, tap)

                    if rnd == NRND - 1:
                        # per-bank evict (split ACT/DVE) + per-bank store,
                        # DMA issues alternating Sync/GpSimd so the in-order
                        # issue cost doesn't serialize the tail. The final
                        # bank splits in two so its evict and store each run
                        # on both engines/queues in parallel.
                        for g in range(7):
                            dst = osb[:, g * GRP_PIX : (g + 1) * GRP_PIX]
                            if g % 2 == 0:
                                nc.scalar.activation(dst, ps[g][:], Copy, scale=OSCALE)
                            else:
                                nc.vector.tensor_scalar_mul(dst, ps[g][:], OSCALE)
                            eng = nc.sync if g % 2 == 0 else nc.gpsimd
                            eng.dma_start(
                                out=out[
                                    b,
                                    ch * 128 : (ch + 1) * 128,
                                    g * GRP_PIX : (g + 1) * GRP_PIX,
                                ],
                                in_=dst,
                            )
                        HGP = GRP_PIX // 2  # 196
                        p0 = 7 * GRP_PIX
                        for h, eng_e, eng_d in (
                            (0, nc.scalar, nc.sync),
                            (1, nc.vector, nc.gpsimd),
                        ):
                            lo, hi = p0 + h * HGP, p0 + (h + 1) * HGP
                            dst = osb[:, lo:hi]
                            if h == 0:
                                eng_e.activation(dst, ps[7][:, 0:HGP], Copy, scale=OSCALE)
                            else:
                                eng_e.tensor_scalar_mul(dst, ps[7][:, HGP:GRP_PIX], OSCALE)
                            eng_d.dma_start(
                                out=out[b, ch * 128 : (ch + 1) * 128, lo:hi],
                                in_=dst,
                            )
                    else:
                        for g in range(8):
                            dst = osb[:, g * GRP_PIX : (g + 1) * GRP_PIX]
                            if g % 2 == 0:
                                nc.scalar.activation(dst, ps[g][:], Copy, scale=OSCALE)
                            else:
                                nc.vector.tensor_scalar_mul(dst, ps[g][:], OSCALE)
                        nc.sync.dma_start(
                            out=out[b, ch * 128 : (ch + 1) * 128, :],
                            in_=osb[:],
                        )
    nc.compile()
    return nc


def _get_nc():
    if "nc" not in _cache:
        _cache["nc"] = _build()
    return _cache["nc"]


def _maybe_install_trace_bridge():
    """Optional: bridge antenv.axon_hooks so trace=True can capture NTFF."""
    import sys
    import types

    if "antenv.axon_hooks" in sys.modules:
        return
    try:
        from trn_agent_boot.trn_boot import _ntff_profile_via_ctypes

        hook = _ntff_profile_via_ctypes("/opt/axon/libaxon_pjrt.so")
        mod = types.ModuleType("antenv.axon_hooks")
        mod.get_axon_ntff_profile_hook = lambda: hook
        mod.set_axon_ntff_profile_hook = lambda h: None
        import antenv

        sys.modules["antenv.axon_hooks"] = mod
        antenv.axon_hooks = mod
    except Exception:
        pass


def kernel(**inputs):
    import os

    from concourse.bass_utils import run_bass_kernel_spmd

    x = np.ascontiguousarray(np.asarray(inputs["x"], dtype=np.float32))
    weight = np.ascontiguousarray(np.asarray(inputs["weight"], dtype=np.float32))
    assert x.shape == (B, CIN, H, W), x.shape
    assert weight.shape == (COUT, CIN, 3, 3), weight.shape

    # [Cout, Cin, kh, kw] -> [Cin, (ch, kh kw, co128)] so each (ch, tap)
    # slice is a ready [K=ci, M=co] stationary operand, ch-major so the
    # kernel can stage the ch=0 half first.
    w_r = np.ascontiguousarray(
        weight.reshape(2, 128, CIN, 9)
        .transpose(2, 0, 3, 1)
        .reshape(CIN, 9 * COUT)
    )
    xp = np.zeros((B, CIN, HP, HP), dtype=np.float32)
    xp[:, :, 1 : 1 + H, 1 : 1 + W] = x.reshape(B, CIN, H, W)
    xp = xp.reshape(B, CIN, NPAD)
    in_maps = [
        {"x": xp[i * BL : (i + 1) * BL], "w": w_r}
        for i in range(NCORES)
    ]

    trace = bool(int(os.environ.get("KERNEL_TRACE", "0")))
    if trace:
        _maybe_install_trace_bridge()
    nc = _get_nc()
    res = run_bass_kernel_spmd(nc, in_maps, core_ids=list(range(NCORES)), trace=trace)
    _cache["exec_time_ns"] = res.exec_time_ns
    _cache["res"] = res

    outs = [res.results[i]["out"].reshape(BL, COUT, H, W) for i in range(NCORES)]
    return np.concatenate(outs, axis=0)


# revision 11
# speedup vs baseline: 1.1805x; 1.0812x over previous
"""Trainium2 Bass kernel for quantized 3x3 conv2d (stride 1, pad 1).

Reference computes: conv2d(quant16(x), quant16(w)) where quant16 rounds to
signed 16-bit fixed point with 12 fractional bits (round-half-even, /4096).

Strategy (per core, data-parallel over batch: 4 images/core on 8 cores):

  1D Winograd F(2,3) along H in GEMM form. For output row pair (2t, 2t+1),
  with d_k = padded input row 2t+k and vertical taps g0,g1,g2:
      v0 = d0-d2   v1 = d1+d2   v2 = d2-d1   v3 = d1-d3          (DVE)
      m_k = sum_dw  Wk(dw) @ vk(shifted by dw)                   (PE, PSUM)
      W0 = g0*s,  W1 = (g0+g1+g2)*s/2,  W2 = (g0-g1+g2)*s/2,  W3 = g2*s
      y(2t)   = m0+m1+m2                                         (DVE)
      y(2t+1) = m1-m2-m3                                         (DVE)
  12 matmul passes per (img, cout-chunk, row-chunk) vs 18 for direct conv:
  PE time drops by 1/3. The 2^-24 fixed-point descale folds into the
  transformed weights (s = 2^-23, x carries the other 2^-1), so PSUM holds
  final-scale values and the combine needs no extra scaling pass.

  Quantization: magic-number trick (+1.5*2^23 in f32 RNE) gives
  rx = round(x*4096) exactly; xh5 = fp16(rx/2) (~2^-12 rel err). Weights
  quantize to exact fp16 integers, transform on-device on DVE. End-to-end
  max rel err ~3.5e-4 vs the 2e-2 gate (CPU-verified).

  Layout: padded 58x58 image as [Cin=128 partitions, 58*58]; rows viewed as
  29 (pair, 2) groups so d0..d3 slice without strided stepping. v planes
  [Cin, (k, t=28, col=58)] fp16. PSUM: 4 m-banks per round, ping-pong on
  round parity. Cout=256 -> two 128-row chunks (ch-major rounds so ch1's
  weight transforms hide under ch0's rounds).
"""

import numpy as np

B, CIN, COUT, H, W = 32, 128, 256, 56, 56
NCORES = 8
BL = B // NCORES          # images per core
HP = H + 2                # padded height/width (58)
NPIX = H * W              # 3136
NPAD = HP * HP            # 3364
SCALE = 4096.0
MAGIC = 12582912.0        # 1.5 * 2**23: f32 add forces round-to-nearest-even at ulp=1
WSC = 2.0 ** -23          # weight scale: (rx/2) * (rw*2^-23) = rx*rw*2^-24
NT = 28                   # tile-rows (output row pairs)
TCH = 7                   # tile-rows per round chunk
NCHK = NT // TCH          # 4 chunks
CHUNK_PIX = TCH * 2 * W   # 784 output px per chunk
VCOLS = 4 * NT * HP       # v-plane columns: (k, t, col)

_cache = {}


def _build():
    import concourse.bacc as bacc
    import concourse.mybir as mybir
    import concourse.tile as tile

    f32, f16 = mybir.dt.float32, mybir.dt.float16
    Copy = mybir.ActivationFunctionType.Copy
    Alu = mybir.AluOpType

    nc = bacc.Bacc("TRN2", target_bir_lowering=False)
    x_in = nc.dram_tensor("x", [BL, CIN, NPAD], f32, kind="ExternalInput")
    w_in = nc.dram_tensor("w", [CIN, 9 * COUT], f32, kind="ExternalInput")
    out = nc.dram_tensor("out", [BL, COUT, NPIX], f32, kind="ExternalOutput")

    HW_COLS = 9 * 128  # 1152 weight columns per cout-half

    with tile.TileContext(nc) as tc:
        with (
            tc.tile_pool(name="fixed", bufs=1) as fx,
            tc.tile_pool(name="psum", bufs=1, space="PSUM") as pp,
        ):
            # ---- per-image ping-pong buffers ----
            xsts = [fx.tile([CIN, NPAD], f32, name=f"xst{i}") for i in range(2)]
            ts = [fx.tile([CIN, NPAD], f32, name=f"t{i}") for i in range(2)]
            xhs = [fx.tile([CIN, NPAD], f16, name=f"xh{i}") for i in range(2)]
            vs = [fx.tile([CIN, VCOLS], f16, name=f"v{i}") for i in range(2)]
            osbs = [fx.tile([128, CHUNK_PIX], f32, name=f"osb{i}") for i in range(3)]
            tmps = [fx.tile([128, TCH * W], f32, name=f"tmp{i}") for i in range(8)]
            ps = [pp.tile([128, TCH * W], f32, name=f"ps{i}") for i in range(8)]
            wst = fx.tile([CIN, 9 * COUT], f32)
            wt = fx.tile([CIN, 9 * COUT], f32)
            w16 = fx.tile([CIN, 9 * COUT], f16)
            # transformed weights [ci, (ch, dw, k, co)]
            wtr = fx.tile([CIN, 2 * 3 * 4 * 128], f16)
            wsc1 = fx.tile([CIN, 128], f16)  # scratch g0+g2
            wsc2 = fx.tile([CIN, 128], f16)  # scratch sums

            def stage_slice(b, r0, r1):
                """DMA a padded-row slice, quantize: t = rx+MAGIC, xh = fp16(rx/2)."""
                s = b % 2
                lo, hi = r0 * HP, r1 * HP
                nc.gpsimd.dma_start(out=xsts[s][:, lo:hi], in_=x_in[b, :, lo:hi])
                nc.scalar.activation(
                    ts[s][:, lo:hi], xsts[s][:, lo:hi], Copy, bias=MAGIC, scale=SCALE
                )
                # xh5 = (t - MAGIC)/2 = rx/2, exact in f32, fp16 on write
                nc.scalar.activation(
                    xhs[s][:, lo:hi], ts[s][:, lo:hi], Copy, bias=-MAGIC / 2, scale=0.5
                )

            def quant_w(lo, hi):
                nc.vector.tensor_scalar(
                    out=wt[:, lo:hi], in0=wst[:, lo:hi],
                    scalar1=SCALE, scalar2=MAGIC, op0=Alu.mult, op1=Alu.add,
                )
                nc.vector.tensor_scalar_add(w16[:, lo:hi], wt[:, lo:hi], -MAGIC)

            def wslice(ch, tap):
                c0 = ch * HW_COLS + tap * 128
                return w16[:, c0 : c0 + 128]

            def wtr_slice(ch, dw, k):
                c0 = ((ch * 3 + dw) * 4 + k) * 128
                return wtr[:, c0 : c0 + 128]

            def transform_w(ch):
                """Per dw: W0 = g0*s, W1 = (g0+g1+g2)*s/2, W2 = (g0-g1+g2)*s/2,
                W3 = g2*s. g sums stay exact/near-exact in fp16; the *s is a
                power-of-two scale (exact)."""
                for dw in range(3):
                    g0, g1, g2 = (wslice(ch, dh * 3 + dw) for dh in range(3))
                    nc.vector.tensor_scalar_mul(wtr_slice(ch, dw, 0), g0, WSC)
                    nc.vector.tensor_tensor(wsc1[:], g0, g2, Alu.add)
                    nc.vector.tensor_tensor(wsc2[:], wsc1[:], g1, Alu.add)
                    nc.vector.tensor_scalar_mul(wtr_slice(ch, dw, 1), wsc2[:], WSC / 2)
                    nc.vector.tensor_tensor(wsc2[:], wsc1[:], g1, Alu.subtract)
                    nc.vector.tensor_scalar_mul(wtr_slice(ch, dw, 2), wsc2[:], WSC / 2)
                    nc.vector.tensor_scalar_mul(wtr_slice(ch, dw, 3), g2, WSC)

            def v_ops(b, tc_i):
                """v planes for tile-rows [7*tc_i, 7*tc_i+7): rows as (pair, 2)
                so d_k are plain slices."""
                s = b % 2
                xh4 = xhs[s][:].rearrange("p (t two c) -> p t two c", two=2, c=HP)
                v4 = vs[s][:].rearrange("p (k t c) -> p k t c", k=4, t=NT)
                t0 = tc_i * TCH
                d0 = xh4[:, t0 : t0 + TCH, 0, :]
                d1 = xh4[:, t0 : t0 + TCH, 1, :]
                d2 = xh4[:, t0 + 1 : t0 + TCH + 1, 0, :]
                d3 = xh4[:, t0 + 1 : t0 + TCH + 1, 1, :]
                nc.vector.tensor_tensor(v4[:, 0, t0 : t0 + TCH, :], d0, d2, Alu.subtract)
                nc.vector.tensor_tensor(v4[:, 1, t0 : t0 + TCH, :], d1, d2, Alu.add)
                nc.vector.tensor_tensor(v4[:, 2, t0 : t0 + TCH, :], d2, d1, Alu.subtract)
                nc.vector.tensor_tensor(v4[:, 3, t0 : t0 + TCH, :], d1, d3, Alu.subtract)

            # ---- head staging: w ch0 first (gates first LDWEIGHTS), x on
            # the GpSimd queue, everything else behind ----
            nc.sync.dma_start(out=wst[:, 0:HW_COLS], in_=w_in[:, 0:HW_COLS])
            stage_slice(0, 0, 16)
            quant_w(0, HW_COLS)
            # PE warmup on raw quantized weights while x/v are still staging
            for _ in range(4):
                nc.tensor.matmul(
                    ps[7][:, 0:384], w16[:, 0:128], w16[:, 0:384],
                    start=True, stop=True,
                )
            transform_w(0)
            v_ops(0, 0)
            stage_slice(0, 16, 30)
            nc.sync.dma_start(
                out=wst[:, HW_COLS : 2 * HW_COLS], in_=w_in[:, HW_COLS : 2 * HW_COLS]
            )
            stage_slice(0, 30, 44)
            stage_slice(0, 44, HP)
            quant_w(HW_COLS, 2 * HW_COLS)
            transform_w(1)
            stage_slice(1, 0, 16)
            stage_slice(1, 16, 30)
            stage_slice(1, 30, 44)
            stage_slice(1, 44, HP)

            NRND = BL * 2 * NCHK
            rnd = 0
            for b in range(BL):
                s = b % 2
                v4 = vs[s][:].rearrange("p (k t c) -> p k t c", k=4, t=NT)
                if b >= 2:
                    for r0, r1 in ((0, 16), (16, 30), (30, 44), (44, HP)):
                        stage_slice(b, r0, r1)
                for ch in range(2):
                    for tc_i in range(NCHK):
                        # v planes are shared by both ch; compute on ch0 pass
                        # (image 0 chunk 0 is emitted in the head)
                        if ch == 0 and (tc_i > 0 or b > 0):
                            v_ops(b, tc_i)
                        bank = (rnd % 2) * 4
                        t0 = tc_i * TCH
                        for k in range(4):
                            for dw in range(3):
                                nc.tensor.matmul(
                                    ps[bank + k][:],
                                    wtr_slice(ch, dw, k),
                                    v4[:, k, t0 : t0 + TCH, dw : dw + W],
                                    start=(dw == 0),
                                    stop=(dw == 2),
                                )
                        # combine: y0 = m0+m1+m2 (even rows), y1 = m1-m2-m3.
                        # DVE can read at most one PSUM operand per op, so
                        # ACT (idle) evicts m1, m2 to SBUF first; then
                        # y0 = m0 + (e1+e2), y1 = (e1-e2) - m3.
                        osb = osbs[rnd % 3]
                        o4 = osb[:].rearrange("p (t r c) -> p t r c", t=TCH, r=2)
                        e1, e2, s12, d12 = (tmps[4 * (rnd % 2) + j] for j in range(4))
                        nc.scalar.activation(e1[:], ps[bank + 1][:], Copy)
                        nc.scalar.activation(e2[:], ps[bank + 2][:], Copy)
                        nc.vector.tensor_tensor(s12[:], e1[:], e2[:], Alu.add)
                        nc.vector.tensor_tensor(d12[:], e1[:], e2[:], Alu.subtract)
                        s12v = s12[:].rearrange("p (t c) -> p t c", t=TCH)
                        d12v = d12[:].rearrange("p (t c) -> p t c", t=TCH)
                        m0v = ps[bank + 0][:].rearrange("p (t c) -> p t c", t=TCH)
                        m3v = ps[bank + 3][:].rearrange("p (t c) -> p t c", t=TCH)
                        nc.vector.tensor_tensor(o4[:, :, 0, :], m0v, s12v, Alu.add)
                        nc.vector.tensor_tensor(o4[:, :, 1, :], d12v, m3v, Alu.subtract)
                        nc.sync.dma_start(
                            out=out[
                                b,
                                ch * 128 : (ch + 1) * 128,
                                tc_i * CHUNK_PIX : (tc_i + 1) * CHUNK_PIX,
                            ],
                            in_=osb[:],
                        )
                        rnd += 1
    nc.compile()
    return nc


def _get_nc():
    if "nc" not in _cache:
        _cache["nc"] = _build()
    return _cache["nc"]


def _maybe_install_trace_bridge():
    """Optional: bridge antenv.axon_hooks so trace=True can capture NTFF."""
    import sys
    import types

    if "antenv.axon_hooks" in sys.modules:
        return
    try:
        from trn_agent_boot.trn_boot import _ntff_profile_via_ctypes

        hook = _ntff_profile_via_ctypes("/opt/axon/libaxon_pjrt.so")
        mod = types.ModuleType("antenv.axon_hooks")
        mod.get_axon_ntff_profile_hook = lambda: hook
        mod.set_axon_ntff_profile_hook = lambda h: None
        import antenv

        sys.modules["antenv.axon_hooks"] = mod
        antenv.axon_hooks = mod
    except Exception:
        pass


def kernel(**inputs):
    import os

    from concourse.bass_utils import run_bass_kernel_spmd

    x = np.ascontiguousarray(np.asarray(inputs["x"], dtype=np.float32))
    weight = np.ascontiguousarray(np.asarray(inputs["weight"], dtype=np.float32))
    assert x.shape == (B, CIN, H, W), x.shape
    assert weight.shape == (COUT, CIN, 3, 3), weight.shape

    # [Cout, Cin, kh, kw] -> [Cin, (ch, kh kw, co128)] so each (ch, tap)
    # slice is a ready [K=ci, M=co] stationary operand, ch-major so the
    # kernel can stage the ch=0 half first.
    w_r = np.ascontiguousarray(
        weight.reshape(2, 128, CIN, 9)
        .transpose(2, 0, 3, 1)
        .reshape(CIN, 9 * COUT)
    )
    xp = np.zeros((B, CIN, HP, HP), dtype=np.float32)
    xp[:, :, 1 : 1 + H, 1 : 1 + W] = x.reshape(B, CIN, H, W)
    xp = xp.reshape(B, CIN, NPAD)
    in_maps = [
        {"x": xp[i * BL : (i + 1) * BL], "w": w_r}
        for i in range(NCORES)
    ]

    trace = bool(int(os.environ.get("KERNEL_TRACE", "0")))
    if trace:
        _maybe_install_trace_bridge()
    nc = _get_nc()
    res = run_bass_kernel_spmd(nc, in_maps, core_ids=list(range(NCORES)), trace=trace)
    _cache["exec_time_ns"] = res.exec_time_ns
    _cache["res"] = res

    outs = [res.results[i]["out"].reshape(BL, COUT, H, W) for i in range(NCORES)]
    return np.concatenate(outs, axis=0)
